# revision 16
# baseline (speedup 1.0000x reference)
"""Trainium2 Bass kernel for nn_Losses_4784593568314 (SILog + bins-chamfer + minmax loss).

Sharding: data-parallel over batch B=8 -> one sample per NeuronCore (8 cores).
Each core computes partial scalars (silog sums, chamfer mins/sums, min/max);
host gathers the 8 partial vectors and combines them into the final scalar.

Device algorithm per core (sample b, P=69312 pixels, 256 bin centers):
  - phase A ([114,608] layout): logs, masks, masked sums for SILog; min/max of
    raw depth; sentinel-ized pixel vector t_x = (d>=eps ? d : 4.0).
  - chamfer: PE computes all pairwise diffs (t - c) via K=4 bf16 matmuls
    (hi/lo split of t and c recovers ~fp32 accuracy; products are exact since
    one factor is +-1, accumulation is fp32 in PSUM).
      x-pass: out[bin, pix]  -> VE reduce(min, |.|) over pixels  -> cham_x
      y-pass: out[pix, bin]  -> VE reduce(min, |.|) over bins    -> cham_y
    Invalid/pad pixels carry sentinel 4.0 so their best |t-c| >= 3, which
    never wins an x-min and is masked out of the y-sum via (min < 3).
"""

import os
import sys
from contextlib import ExitStack

for _p in ("/opt/trn_rl_repo", "/root/.axon_site/_ro/trn_rl_repo"):
    if os.path.isdir(_p) and _p not in sys.path:
        sys.path.insert(0, _p)

import numpy as np

import concourse.bass as bass
import concourse.tile as tile
from concourse import bacc, mybir
from concourse.bass_utils import run_bass_kernel_spmd

AF = mybir.ActivationFunctionType
ALU = mybir.AluOpType
AX = mybir.AxisListType
DT = mybir.dt

NCORES = 8
EPS = 0.01
SENT = 4.0  # sentinel value for invalid/pad pixels (|SENT - c| >= 3 for c in [0,1))
LAMB = 0.85
ALPHA, BETA, GAMMA = 10.0, 0.1, 0.1

P_PIX = 228 * 304  # 69312
PA_P, PA_F = 114, 608  # phase-A layout, 114*608 = 69312
HALF = 34816  # padded pixels per half = 272*128 = 68*512
REAL = 34656  # real pixels per half = 57*608
PAD = HALF - REAL  # 160 sentinel pad pixels per half


def _body(ctx, tc, out_h, o_h, d_h, c_h):
    nc = tc.nc
    f32, bf16 = DT.float32, DT.bfloat16

    singles = ctx.enter_context(tc.tile_pool(name="singles", bufs=1))
    psum = ctx.enter_context(tc.tile_pool(name="psum", bufs=2, space="PSUM"))

    # ---------------- input loads ----------------
    o114 = singles.tile([PA_P, PA_F], f32)
    d114 = singles.tile([PA_P, PA_F], f32)
    c_sb = singles.tile([1, 256], f32)
    nc.gpsimd.dma_start(out=o114[:, :], in_=o_h)
    nc.gpsimd.dma_start(out=d114[:, :], in_=d_h)
    nc.gpsimd.dma_start(out=c_sb[:, :], in_=c_h)

    # ---------------- phase A: silog sums, min/max, sentinel pixels ----------------
    # pre-add EPS on VE so each ACT Ln depends on exactly one engine (the ACT
    # instruction struct carries only a single sync wait)
    lo = singles.tile([PA_P, PA_F], f32)
    ld = singles.tile([PA_P, PA_F], f32)
    oe = singles.tile([PA_P, PA_F], f32)
    de = singles.tile([PA_P, PA_F], f32)
    nc.vector.tensor_scalar(oe[:, :], o114[:, :], EPS, None, ALU.add)
    nc.vector.tensor_scalar(de[:, :], d114[:, :], EPS, None, ALU.add)
    nc.scalar.activation(lo[:, :], oe[:, :], AF.Ln)
    nc.scalar.activation(ld[:, :], de[:, :], AF.Ln)

    mo = singles.tile([PA_P, PA_F], f32)
    md = singles.tile([PA_P, PA_F], f32)
    nc.vector.tensor_scalar(mo[:, :], o114[:, :], EPS, None, ALU.is_ge)
    nc.vector.tensor_scalar(md[:, :], d114[:, :], EPS, None, ALU.is_ge)

    mask = singles.tile([PA_P, PA_F], f32)
    ncol = singles.tile([PA_P, 1], f32)
    nc.vector.tensor_tensor(mask[:, :], mo[:, :], md[:, :], ALU.mult)
    nc.vector.tensor_reduce(ncol[:, :], mask[:, :], AX.X, ALU.add)

    g = singles.tile([PA_P, PA_F], f32)
    nc.vector.tensor_tensor(g[:, :], lo[:, :], ld[:, :], ALU.subtract)
    gm = singles.tile([PA_P, PA_F], f32)
    sgcol = singles.tile([PA_P, 1], f32)
    nc.vector.tensor_tensor(gm[:, :], g[:, :], mask[:, :], ALU.mult)
    nc.vector.tensor_reduce(sgcol[:, :], gm[:, :], AX.X, ALU.add)
    g2 = singles.tile([PA_P, PA_F], f32)
    sg2col = singles.tile([PA_P, 1], f32)
    nc.vector.tensor_tensor(g2[:, :], gm[:, :], gm[:, :], ALU.mult)
    nc.vector.tensor_reduce(sg2col[:, :], g2[:, :], AX.X, ALU.add)

    dmincol = singles.tile([PA_P, 1], f32)
    dmaxcol = singles.tile([PA_P, 1], f32)
    nc.vector.tensor_reduce(dmincol[:, :], d114[:, :], AX.X, ALU.min)
    nc.vector.tensor_reduce(dmaxcol[:, :], d114[:, :], AX.X, ALU.max)

    # t_x = max(d, 4*[d < eps])  (= d where valid, 4.0 where invalid)
    u = singles.tile([PA_P, PA_F], f32)
    nc.vector.tensor_scalar(u[:, :], d114[:, :], EPS, SENT, ALU.is_lt, ALU.mult)
    tx = singles.tile([PA_P, PA_F], f32)
    nc.vector.tensor_tensor(tx[:, :], d114[:, :], u[:, :], ALU.max)
    # hi/lo bf16 split of t_x
    th = singles.tile([PA_P, PA_F], bf16)
    tl = singles.tile([PA_P, PA_F], bf16)
    nc.vector.tensor_copy(th[:, :], tx[:, :])
    nc.vector.tensor_tensor(tl[:, :], tx[:, :], th[:, :], ALU.subtract)

    # ---------------- T3: PE operand layout ----------------
    # partition groups at matmul-legal bases 0 and 32:
    #   base+0 = t_hi, base+1 = t_lo, base+2 = -1, base+3 = -1
    T3 = singles.tile([36, HALF], bf16)
    # fill the constant -1 rows via DMA broadcast from a small staged row
    # (keeps the fill off the critical VectorE)
    neg1 = singles.tile([1, 512], bf16)
    nc.vector.memset(neg1[0:1, :], -1.0)
    n1b = bass.AP(tensor=neg1.tensor, offset=neg1.offset,
                  ap=[[1, 1], [0, 2 * (HALF // 512)], [1, 512]])
    nc.gpsimd.dma_start(out=T3[2:4, :], in_=n1b)
    nc.gpsimd.dma_start(out=T3[34:36, :], in_=n1b)
    # rows 0,1 / 32,33 are fully overwritten by the t_hi/t_lo DMAs + pad fills
    # pad fills staged through a single-row tile (compute engines must start
    # at partition 0/32/64/96; DMA has no such restriction)
    padv = singles.tile([1, 2 * PAD], bf16)
    nc.vector.memset(padv[0:1, 0:PAD], SENT)
    nc.vector.memset(padv[0:1, PAD:2 * PAD], 0.0)
    nc.gpsimd.dma_start(out=T3[0:1, REAL:HALF], in_=padv[0:1, 0:PAD])
    nc.gpsimd.dma_start(out=T3[1:2, REAL:HALF], in_=padv[0:1, PAD:2 * PAD])
    nc.gpsimd.dma_start(out=T3[32:33, REAL:HALF], in_=padv[0:1, 0:PAD])
    nc.gpsimd.dma_start(out=T3[33:34, REAL:HALF], in_=padv[0:1, PAD:2 * PAD])
    nc.gpsimd.dma_start(out=T3[0:1, 0:REAL], in_=th[0:57, :])
    nc.gpsimd.dma_start(out=T3[1:2, 0:REAL], in_=tl[0:57, :])
    nc.gpsimd.dma_start(out=T3[32:33, 0:REAL], in_=th[57:114, :])
    nc.gpsimd.dma_start(out=T3[33:34, 0:REAL], in_=tl[57:114, :])

    # cw4: rows base+0=ones base+1=ones base+2=c_hi base+3=c_lo, duplicated at
    # bases 0 and 32 (matmul requires lhsT/rhs to share base partition)
    cw4 = singles.tile([36, 256], bf16)
    nc.vector.memset(cw4[0:2, :], 1.0)
    nc.vector.memset(cw4[32:34, :], 1.0)
    chi = singles.tile([1, 256], bf16)
    clo = singles.tile([1, 256], bf16)
    nc.vector.tensor_copy(chi[:, :], c_sb[:, :])
    nc.vector.tensor_tensor(clo[:, :], c_sb[:, :], chi[:, :], ALU.subtract)
    nc.gpsimd.dma_start(out=cw4[2:3, :], in_=chi[:, :])
    nc.gpsimd.dma_start(out=cw4[3:4, :], in_=clo[:, :])
    nc.gpsimd.dma_start(out=cw4[34:35, :], in_=chi[:, :])
    nc.gpsimd.dma_start(out=cw4[35:36, :], in_=clo[:, :])

    # ---------------- chamfer x-pass: per-bin min over pixels ----------------
    # out[bin, pix] = t_pix - c_bin ; reduce min(|.|) over pixel chunks
    xmins = singles.tile([128, 2, 34], f32)
    for bh in range(2):
        for ph in range(2):
            gb = 32 * ph
            lhsT = cw4[gb:gb + 4, bh * 128:(bh + 1) * 128]
            for s in range(17):
                ps = psum.tile([128, 4, 512], f32, tag="ps")
                base = s * 2048
                for q in range(4):
                    nc.tensor.matmul(
                        ps[:, q, :], lhsT,
                        T3[gb:gb + 4, base + q * 512: base + (q + 1) * 512])
                nc.vector.tensor_reduce(
                    xmins[:, bh, ph * 17 + s: ph * 17 + s + 1], ps[:, :, :],
                    AX.XY, ALU.min, apply_absolute_value=True)

    # ---------------- chamfer y-pass: per-pixel min over bins ----------------
    # out[pix, bin] = t_pix - c_bin ; reduce min(|.|) over the 256 bins
    ymins = singles.tile([128, 544], f32)
    for ph in range(2):
        for chunk in range(34):
            ps = psum.tile([128, 8, 256], f32, tag="ps")
            gb = 32 * ph
            for i in range(8):
                j = chunk * 8 + i
                nc.tensor.matmul(
                    ps[:, i, :],
                    T3[gb:gb + 4, j * 128:(j + 1) * 128],
                    cw4[gb:gb + 4, 0:256])
            c0 = (ph * 34 + chunk) * 8
            nc.vector.tensor_reduce(
                ymins[:, c0:c0 + 8], ps[:, :, :],
                AX.X, ALU.min, apply_absolute_value=True)

    # ---------------- finals ----------------
    ymask = singles.tile([128, 544], f32)
    nc.vector.tensor_scalar(ymask[:, :], ymins[:, :], 3.0, None, ALU.is_lt)
    ym = singles.tile([128, 544], f32)
    nc.vector.tensor_tensor(ym[:, :], ymins[:, :], ymask[:, :], ALU.mult)
    ym2 = singles.tile([128, 544], f32)
    miny2col = singles.tile([128, 1], f32)
    nc.vector.tensor_tensor(ym2[:, :], ym[:, :], ym[:, :], ALU.mult)
    nc.vector.tensor_reduce(miny2col[:, :], ym2[:, :], AX.X, ALU.add)
    nvcol = singles.tile([128, 1], f32)
    nc.vector.tensor_reduce(nvcol[:, :], ymask[:, :], AX.X, ALU.add)

    xm = singles.tile([128, 2], f32)
    nc.vector.tensor_reduce(xm[:, :], xmins[:, :, :], AX.X, ALU.min)
    xm2 = singles.tile([128, 2], f32)
    nc.vector.tensor_tensor(xm2[:, :], xm[:, :], xm[:, :], ALU.mult)

    # partial-scalar block [128, 10]: cols 0-7 additive, cols 8-9 max-reduced.
    # Cross-partition reduction is done by DMA-flattening the block to one
    # partition row and reducing with a stride-permuted AP on VE.
    blk = singles.tile([128, 10], f32)
    nc.vector.memset(blk[:, 0:8], 0.0)
    nc.vector.memset(blk[:, 8:10], -1e30)
    nc.vector.tensor_copy(blk[0:PA_P, 0:1], ncol[:, :])
    nc.vector.tensor_copy(blk[0:PA_P, 1:2], sgcol[:, :])
    nc.vector.tensor_copy(blk[0:PA_P, 2:3], sg2col[:, :])
    nc.vector.tensor_copy(blk[:, 3:4], miny2col[:, :])
    nc.vector.tensor_copy(blk[:, 4:5], nvcol[:, :])
    nc.vector.tensor_copy(blk[:, 5:7], xm2[:, :])
    negdmin = singles.tile([PA_P, 1], f32)
    nc.vector.tensor_scalar(negdmin[:, :], dmincol[:, :], -1.0, None, ALU.mult)
    nc.vector.tensor_copy(blk[0:PA_P, 8:9], negdmin[:, :])
    nc.vector.tensor_copy(blk[0:PA_P, 9:10], dmaxcol[:, :])

    row = singles.tile([1, 1280], f32)
    nc.gpsimd.dma_start(out=row[0:1, :], in_=blk[:, :])
    # row element (p, c) at offset p*10 + c; view as [1, c, p] to reduce over p
    rview = row[0:1, :].rearrange("a (p c) -> a c p", c=10)
    outt = singles.tile([1, 16], f32)
    nc.vector.memset(outt[:, :], 0.0)
    nc.vector.tensor_reduce(outt[0:1, 0:8], rview[:, 0:8, :], AX.X, ALU.add)
    nc.vector.tensor_reduce(outt[0:1, 8:10], rview[:, 8:10, :], AX.X, ALU.max)
    nc.gpsimd.dma_start(out=out_h, in_=outt[:, :])


def build_module():
    nc = bacc.Bacc("TRN2", target_bir_lowering=False, debug=False, num_devices=NCORES)
    o_h = nc.dram_tensor("o", [PA_P, PA_F], DT.float32, kind="ExternalInput").ap()
    d_h = nc.dram_tensor("d", [PA_P, PA_F], DT.float32, kind="ExternalInput").ap()
    c_h = nc.dram_tensor("c", [1, 256], DT.float32, kind="ExternalInput").ap()
    out_h = nc.dram_tensor("partials", [1, 16], DT.float32, kind="ExternalOutput").ap()
    with tile.TileContext(nc) as tc:
        with ExitStack() as ctx:
            _body(ctx, tc, out_h, o_h, d_h, c_h)
    nc.compile()
    return nc


_CACHE = {}


def _get_module():
    if "nc" not in _CACHE:
        _CACHE["nc"] = build_module()
    return _CACHE["nc"]


def _combine(parts, epoch, centers):
    """parts: [8, 16] float64 partial vectors; returns final loss (float)."""
    n = parts[:, 0].sum()
    sg = parts[:, 1].sum()
    sg2 = parts[:, 2].sum()
    mean_g = sg / n
    var_g = (sg2 - n * mean_g * mean_g) / (n - 1.0)
    sil = np.sqrt(var_g + (1.0 - LAMB) * mean_g * mean_g)

    cham_x = ((parts[:, 5] + parts[:, 6]) / 256.0).mean()
    cham_y = (parts[:, 3] / parts[:, 4]).mean()
    bc = cham_x + cham_y

    dmin = -parts[:, 8]
    dmax = parts[:, 9]
    c64 = np.asarray(centers, np.float64)
    mm = np.abs(c64[:, -1] - dmax).sum() + np.abs(c64[:, 0] - dmin).sum()

    loss = ALPHA * sil + BETA * bc
    if int(epoch) >= 10:
        loss = loss + GAMMA * mm
    return loss


def run_on_device(output, centers, depth, trace=False):
    nc = _get_module()
    output = np.asarray(output, np.float32)
    depth = np.asarray(depth, np.float32)
    centers = np.asarray(centers, np.float32)
    in_maps = []
    for b in range(NCORES):
        in_maps.append({
            "o": np.ascontiguousarray(output[b, 0].reshape(PA_P, PA_F)),
            "d": np.ascontiguousarray(depth[b, 0].reshape(PA_P, PA_F)),
            "c": np.ascontiguousarray(centers[b].reshape(1, 256)),
        })
    res = run_bass_kernel_spmd(nc, in_maps, list(range(NCORES)), trace=trace)
    parts = np.stack(
        [res.results[b]["partials"].reshape(-1) for b in range(NCORES)]
    ).astype(np.float64)
    return parts, res


def kernel(epoch, output, centers, depth, lidar):
    parts, _ = run_on_device(output, centers, depth, trace=False)
    loss = _combine(parts, epoch, centers)
    return np.float32(loss)
